# revision 32
# baseline (speedup 1.0000x reference)
"""DiT block kernel for TRN2, 8 NeuronCores, token-parallel sharding.

v2: fp8 DoubleRow matmuls for attention + FFN up-proj, bf16 down-proj,
bf16 PE transposes, host-pretiled weight images (few big DMAs), packed
constants, bias folding (v-bias/cv-bias folded into out-proj bias rows).

Sharding: core c handles batch b=c//2, sequence half c%2 (512 q tokens);
K/V computed over the batch's full 1024 tokens (no collectives).

Layouts:
  xs_pack[s]  [128, 2, S]  fp8  plane dpl=d-tile (2s+dpl), LN1 output
  q_pack[j]   [128, 2, T]  fp8  partition p: head 4j+p//32, feat pl*32+p%32
  k_pack[j]   [128, 2, S]  fp8  same permutation as q
  v_pack[cc]  [128, 2, 16, 65] fp8 ktok-major, c-pair planes, ones col 64
  sa_pack[t]  [128, 2, T]  fp8  plane = d-tile (2t+dpl)
  cq/ck/cross_pack analogous; cv_pack [128, 2, SC*? ] ctok-major planes=c
  h_pack[t]   [128, 2, T]  bf16 plane = ff-tile (2t+dpl)
  xs3 hi e4m3 + lo e5m2 planes for the FFN 3-chain up-proj.
"""

import numpy as np
import ml_dtypes

import concourse.bass as bass
import concourse.bacc as bacc
import concourse.tile as tile
from concourse import mybir
from concourse.masks import make_identity

B, S, SC, D, H = 4, 1024, 256, 1024, 16
HD = D // H          # 64
FF = 4 * D           # 4096
T = 512              # local query tokens per core
P = 128
DT = D // P          # 8
QC = T // P          # 4
KC = S // P          # 8
CC = SC // P         # 2
FT = FF // P         # 32
EPS = 1e-5
N_CORES = 8

f32 = mybir.dt.float32
f32r = mybir.dt.float32r
bf16 = mybir.dt.bfloat16
f8 = mybir.dt.float8e4      # ml_dtypes.float8_e4m3
f8e5 = mybir.dt.float8e5    # ml_dtypes.float8_e5m2
AF = mybir.ActivationFunctionType
ALU = mybir.AluOpType
DR = mybir.MatmulPerfMode.DoubleRow

NP_F8 = ml_dtypes.float8_e4m3
NP_F8E5 = ml_dtypes.float8_e5m2
NP_BF16 = ml_dtypes.bfloat16

# consts_col column indices
QB0, KB0 = 0, 8            # q/k bias (permuted), 8 cols each
B1_0 = 16                  # b1, 32 cols
CQB0, CKB0 = 48, 56        # cq/ck bias, 8 cols each
MSK0 = 64                  # cond mask-mul, 2 cols
TG0 = 66                   # tanh(gate) broadcast col
LN0 = 67                   # 8 tensors x 8 cols: ln1_g,ln1_b,ln2x_g,ln2x_b,
                           #                      ln2c_g,ln2c_b,ln3_g,ln3_b
NCOL = LN0 + 64


def r(ap):
    return ap.bitcast(f32r)


def build_nc(gelu_func=None, compile_hw=False):
    if gelu_func is None:
        gelu_func = AF.Gelu
    nc = bacc.Bacc("TRN2", target_bir_lowering=False, debug=False,
                   num_devices=N_CORES)

    xkv = nc.dram_tensor("xkv", [T, D], f32, kind="ExternalInput")
    xoth = nc.dram_tensor("xoth", [T, D], bf16, kind="ExternalInput")
    cond = nc.dram_tensor("cond", [SC, D], bf16, kind="ExternalInput")
    ccol = nc.dram_tensor("ccol", [P, NCOL], f32, kind="ExternalInput")
    crow = nc.dram_tensor("crow", [1, 3 * D], f32r, kind="ExternalInput")
    # fp8 weight chunks, consumption order (see make_in_maps):
    # c0 = wq|wk   c1 = wv|wso   c2 = wqc|wkc   c3 = wvc|wo
    # c4 = w1hi(f0-15)  c5 = w1lo(f0-15)  c6 = w1hi(f16-31)  c7 = w1lo(f16-31)
    wch = [nc.dram_tensor(f"wch{i}", [P, 8192], f8, kind="ExternalInput")
           for i in range(16)]
    w2ch = [nc.dram_tensor(f"w2ch{i}", [P, 16384], f8, kind="ExternalInput")
            for i in range(4)]

    out = nc.dram_tensor("out", [T, D], f32, kind="ExternalOutput")

    from contextlib import ExitStack
    with tile.TileContext(nc) as tc, ExitStack() as ctx:
        ec = ctx.enter_context
        con = ec(tc.tile_pool(name="con", bufs=1))
        wpool = ec(tc.tile_pool(name="wpool", bufs=6))    # fp8 weight chunks
        w2pool = ec(tc.tile_pool(name="w2pool", bufs=2))  # bf16 w2 chunks
        xsp = ec(tc.tile_pool(name="xsp", bufs=4))        # xs_pack [P,2,S] f8
        khp = ec(tc.tile_pool(name="khp", bufs=16))       # k_pack + h_pack bytes
        qp = ec(tc.tile_pool(name="qp", bufs=8))          # q_pack / cq / sa / cross
        vxp = ec(tc.tile_pool(name="vxp", bufs=4))        # v_pack + xs3 bytes
        cnp = ec(tc.tile_pool(name="cnp", bufs=4))        # cn planes [P,2,SC]
        ckp = ec(tc.tile_pool(name="ckp", bufs=4))        # ck planes
        cvp = ec(tc.tile_pool(name="cvp", bufs=1))        # cv [128,2,1024]
        resid = ec(tc.tile_pool(name="resid", bufs=4))    # x1/x2 [P,D] f32
        xstr = ec(tc.tile_pool(name="xstr", bufs=2))      # residual input stream
        expp = ec(tc.tile_pool(name="expp", bufs=4))      # exp [P,2,T] f8
        lnin = ec(tc.tile_pool(name="lnin", bufs=2))      # LN input stream f32
        lnn = ec(tc.tile_pool(name="lnn", bufs=2))        # normalized bf16
        lsc = ec(tc.tile_pool(name="lsc", bufs=4))        # ln3 f32 scratch [P,128]
        osb = ec(tc.tile_pool(name="osb", bufs=2))        # output staging
        smal = ec(tc.tile_pool(name="smal", bufs=4))      # LN stats
        rcp = ec(tc.tile_pool(name="rcp", bufs=2))        # softmax recip rows
        pa = ec(tc.tile_pool(name="pa", bufs=4, space="PSUM"))   # [P,T] f32
        pb = ec(tc.tile_pool(name="pb", bufs=2, space="PSUM"))   # [P,2,T] f32

        # ---- constants ----
        ident = con.tile([P, P], f32, tag="ident")
        make_identity(nc, ident)
        identb = con.tile([P, P], bf16, tag="identb")
        nc.scalar.activation(identb, ident, AF.Copy)
        ones_f32 = con.tile([P, 1], f32, tag="ones_f32")
        nc.vector.memset(ones_f32, 1.0)
        ones_row = con.tile([1, P], f32r, tag="ones_row")
        nc.scalar.activation(ones_row, ones_f32[0:1, 0:1].to_broadcast([1, P]),
                             AF.Copy)
        ones8 = con.tile([P, 2, 1], f8, tag="ones8")
        nc.scalar.activation(ones8, ones_f32.to_broadcast([P, 2, 1]), AF.Copy)
        eps_t = con.tile([P, 1], f32, tag="eps")
        nc.vector.memset(eps_t, EPS)

        cc_sb = con.tile([P, NCOL], f32, tag="ccol")
        nc.sync.dma_start(out=cc_sb, in_=ccol[:, :])
        cr_sb = con.tile([1, 3 * D], f32r, tag="crow")
        nc.sync.dma_start(out=cr_sb, in_=crow[0:1, :])

        def lncol(idx):
            return (cc_sb[:, LN0 + idx * 8:LN0 + idx * 8 + 8])

        lng = {nm: (lncol(2 * i), lncol(2 * i + 1))
               for i, nm in enumerate(("ln1", "ln2x", "ln2c", "ln3"))}

        tc.strict_bb_all_engine_barrier()

        # weight chunk loads (prefetched in consumption order; wpool bufs=2
        # keeps chunk n+1 in flight while n is consumed)
        wtile = [None] * 16
        w2tile = [None] * 4

        W_FLOOR = {0: 0.018, 1: 0.024, 2: 0.030, 3: 0.040, 4: 0.050,
                   5: 0.054, 6: 0.058, 7: 0.062, 8: 0.080, 9: 0.085,
                   10: 0.090, 11: 0.095, 12: 0.100, 13: 0.105,
                   14: 0.110, 15: 0.115}
        W2_FLOOR = {0: 0.120, 1: 0.130, 2: 0.140, 3: 0.150}

        def loadw(i):
            # legacy call sites pass the old 16KB-chunk index
            for s_ in (2 * i, 2 * i + 1):
                wtile[s_] = wpool.tile([P, 8192], f8, tag="wch",
                                       name=f"wch_{s_}")
                with tc.tile_wait_until(W_FLOOR[s_]):
                    nc.sync.dma_start(out=wtile[s_], in_=wch[s_][:, :])

        def loadw2(i):
            w2tile[i] = w2pool.tile([P, 16384], f8, tag="w2ch",
                                    name=f"w2ch_{i}")
            with tc.tile_wait_until(W2_FLOOR.get(i, 0)):
                nc.sync.dma_start(out=w2tile[i], in_=w2ch[i][:, :])

        def w2v_(t16, dh, lo):
            # [128, 2, 512] DR view; chunk = (hi/lo, dh), content t16-major
            ch = (2 if lo else 0) + dh
            t = w2tile[ch][:, t16 * 1024:t16 * 1024 + 1024]
            if lo:
                t = t.bitcast(f8e5)
            return t.rearrange("p (pl c) -> p pl c", pl=2)

        def wv_(i, off, n):
            # [P, 2, n] DoubleRow view; (i, off) use legacy 16KB indexing
            s_, o_ = 2 * i + off // 8192, off % 8192
            return wtile[s_][:, o_:o_ + 2 * n].rearrange(
                "p (dpl c) -> p dpl c", dpl=2)

        def wv5_(i, off, n):
            s_, o_ = 2 * i + off // 8192, off % 8192
            return wtile[s_][:, o_:o_ + 2 * n].bitcast(f8e5).rearrange(
                "p (dpl c) -> p dpl c", dpl=2)

        loadw(0)

        # ---- helpers ----
        def ln_tile(x_ap, gt, bt, writer, stats_on_act=False):
            """LayerNorm one token-major [P, D] f32 tile; transpose each
            d-tile j to bf16 and call writer(j, psum_bf16_ap).
            stats_on_act routes the two stats passes to ACT (accum_out)
            for windows where DVE is the bottleneck."""
            xn = lnn.tile([P, D], bf16, tag="ln_n")
            mv = smal.tile([P, 2], f32, tag="mv")
            if stats_on_act:
                sums = smal.tile([P, 2], f32, tag="acc")
                nc.scalar.activation(xn, x_ap, AF.Identity,
                                     accum_out=sums[:, 0:1])
                nc.scalar.activation(xn, x_ap, AF.Square,
                                     accum_out=sums[:, 1:2])
                musq = smal.tile([P, 1], f32, tag="musq")
                nc.vector.tensor_scalar_mul(mv[:, 0:1], sums[:, 0:1], 1.0 / D)
                nc.vector.tensor_tensor(musq, mv[:, 0:1], mv[:, 0:1],
                                        ALU.mult)
                nc.vector.scalar_tensor_tensor(mv[:, 1:2], sums[:, 1:2],
                                               1.0 / D, musq,
                                               ALU.mult, ALU.subtract)
            else:
                stats = smal.tile([P, 2, 6], f32, tag="stats")
                nc.vector.bn_stats(out=stats[:, 0, :], in_=x_ap[:, 0:512])
                nc.vector.bn_stats(out=stats[:, 1, :], in_=x_ap[:, 512:1024])
                nc.vector.bn_aggr(out=mv, in_=stats)
            sd = smal.tile([P, 1], f32, tag="sd")
            nc.scalar.activation(sd, mv[:, 1:2], AF.Sqrt, bias=eps_t)
            nc.vector.reciprocal(sd, sd)
            nc.vector.tensor_scalar(xn, x_ap, mv[:, 0:1], sd,
                                    ALU.subtract, ALU.mult)
            for j in range(DT):
                ps_t = pa.tile([P, T], f32, tag="pa", name="ptr")
                pt = ps_t[:, 0:P].bitcast(bf16)[:, 0:P]
                nc.tensor.transpose(pt, xn[:, j * P:(j + 1) * P], identb)
                writer(j, pt, gt[:, j:j + 1], bt[:, j:j + 1])

        # ================= Phase A: LN1(xkv) -> xs_pack =================
        g1, b1t = lng["ln1"]
        xs_pack = [xsp.tile([P, 2, S], f8, tag="xsp", name=f"xs_{s}")
                   for s in range(4)]
        k_packb = [khp.tile([P, 2048], mybir.dt.uint8, tag="khp",
                            name=f"k_{j}") for j in range(4)]
        def ln1_tile(i):
            xtb = lnin.tile([P, 4 * D], mybir.dt.uint8, tag="ln_in")
            if i < QC:
                xt = xtb.bitcast(f32)
                nc.sync.dma_start(out=xt, in_=xkv[i * P:(i + 1) * P, :])
            else:
                xt = xtb[:, 0:2 * D].bitcast(bf16)
                nc.sync.dma_start(out=xt,
                                  in_=xoth[(i - QC) * P:(i - QC + 1) * P, :])

            def wr_xs(j, pt, g_c, b_c, i=i):
                nc.scalar.activation(
                    xs_pack[j // 2][:, j % 2, i * P:(i + 1) * P],
                    pt, AF.Identity, bias=b_c, scale=g_c)
            ln_tile(xt, g1, b1t, wr_xs)

        for i in range(QC):
            ln1_tile(i)
        k_pack = [kpb.bitcast(f8).rearrange("p (pl n) -> p pl n", pl=2)
                  for kpb in k_packb]

        # Q proj only needs the local 512 tokens (tiles 0..3) -> emit early
        q_pack = [qp.tile([P, 2, T], f8, tag="qp", name=f"q_{j}")
                  for j in range(4)]
        for j in range(4):
            for pl in range(2):
                ps = pa.tile([P, T], f32, tag="pa")
                off = (j * 2 + pl) * 1024
                for s in range(4):
                    nc.tensor.matmul(ps, wv_(0, off + s * 256, P),
                                     xs_pack[s][:, :, 0:T],
                                     start=(s == 0), stop=(s == 3),
                                     perf_mode=DR)
                nc.scalar.activation(q_pack[j][:, pl, :], ps, AF.Identity,
                                     bias=cc_sb[:, QB0 + j * 2 + pl:
                                                QB0 + j * 2 + pl + 1])

        # K / V over the LOCAL token half first, so attention heads can
        # begin (scores/AV for ktok tiles 0..3) while LN1 of the other
        # half and cond still run.
        loadw(1)
        loadw(2)

        def k_half(hh):
            for j in range(4):
                for pl in range(2):
                    off = 8192 + (j * 2 + pl) * 1024
                    ps = pa.tile([P, T], f32, tag="pa")
                    for s in range(4):
                        nc.tensor.matmul(ps, wv_(0, off + s * 256, P),
                                         xs_pack[s][:, :, hh * T:(hh + 1) * T],
                                         start=(s == 0), stop=(s == 3),
                                         perf_mode=DR)
                    nc.scalar.activation(k_pack[j][:, pl, hh * T:(hh + 1) * T],
                                         ps, AF.Identity,
                                         bias=cc_sb[:, KB0 + j * 2 + pl:
                                                    KB0 + j * 2 + pl + 1])

        def v_tiles(c_lo, c_hi):
            for c in range(c_lo, c_hi):
                for hh in range(2):
                    ps = pa.tile([P, T], f32, tag="pa")
                    for s in range(4):
                        nc.tensor.matmul(
                            ps, xs_pack[s][:, :, c * P:(c + 1) * P],
                            wv_(1, hh * 4096 + s * 1024, T),
                            start=(s == 0), stop=(s == 3), perf_mode=DR)
                    nc.vector.tensor_copy(
                        out=v_pack[c // 2][:, c % 2, hh * 8:(hh + 1) * 8, 0:HD],
                        in_=ps.rearrange("p (h d) -> p h d", h=8))

        v_packb = [vxp.tile([P, 2080], mybir.dt.uint8, tag="vxp",
                            name=f"v_{cc}") for cc in range(4)]
        v_pack = [t.bitcast(f8).rearrange("p (pl h d) -> p pl h d",
                                          pl=2, h=H) for t in v_packb]
        for cc in range(4):
            nc.gpsimd.tensor_copy(
                out=v_pack[cc][:, :, :, HD:HD + 1],
                in_=ones_f32.to_broadcast([P, 2, H, 1]))

        k_half(0)
        v_tiles(0, 4)

        for i in range(QC, KC):
            ln1_tile(i)

        k_half(1)
        v_tiles(4, 8)

        # cond LN (independent): cn planes
        g2c, b2c = lng["ln2c"]
        cn_pack = [cnp.tile([P, 2, SC], f8, tag="cnp", name=f"cn_{s}")
                   for s in range(4)]
        for i in range(CC):
            ctb = lnin.tile([P, 4 * D], mybir.dt.uint8, tag="ln_in")
            ct = ctb[:, 0:2 * D].bitcast(bf16)
            nc.sync.dma_start(out=ct, in_=cond[i * P:(i + 1) * P, :])

            def wr_cn(j, pt, g_c, b_c, i=i):
                nc.scalar.activation(
                    cn_pack[j // 2][:, j % 2, i * P:(i + 1) * P],
                    pt, AF.Identity, bias=b_c, scale=g_c)
            ln_tile(ct, g2c, b2c, wr_cn)

        # ck / cv from cn planes (chunk2 = wqc|wkc loaded later with cross)
        # ================= Phase C: self-attention =================
        sa_pack = [qp.tile([P, 2, T], f8, tag="qp", name=f"sa_{t}")
                   for t in range(4)]
        for h in range(H):
            j, a = h // 4, h % 4
            ps_av = pa.tile([P, T], f32, tag="pa")
            for cc in range(4):
                psc = pb.tile([P, 2, T], f32, tag="pb")
                for ci in range(2):
                    c = 2 * cc + ci
                    nc.tensor.matmul(
                        psc[:, ci, :],
                        k_pack[j][32 * a:32 * a + 32, :, c * P:(c + 1) * P],
                        q_pack[j][32 * a:32 * a + 32, :, :],
                        start=True, stop=True, perf_mode=DR,
                        tile_position=(32 * a, 0))
                ex = expp.tile([P, 2, T], f8, tag="expp")
                nc.scalar.activation(ex, psc, AF.Exp, scale=0.125)
                nc.tensor.matmul(ps_av[0:HD + 1, :], v_pack[cc][:, :, h, :],
                                 ex, start=(cc == 0), stop=(cc == 3),
                                 perf_mode=DR)
            recip = rcp.tile([1, T], f32r, tag="recip")
            with nc.allow_low_precision(reason="softmax 1/sum in f32r"):
                nc.vector.reciprocal(recip, ps_av[HD:HD + 1, :])
            ps_rb = pa.tile([P, T], f32, tag="pa")
            nc.tensor.matmul(ps_rb[0:HD, :], r(ones_row[0:1, 0:HD]), r(recip),
                             start=True, stop=True)
            rb = rcp.tile([HD, T], f32, tag="rb")
            nc.vector.tensor_copy(out=rb, in_=ps_rb[0:HD, :])
            nc.vector.tensor_tensor(
                sa_pack[h // 4][64 * (h % 2):64 * (h % 2) + 64, (h % 4) // 2, :],
                ps_av[0:HD, :], rb, ALU.mult)

        # out-proj + bsout_eff + residual -> x1
        x1 = [resid.tile([P, D], f32, tag="resid", name=f"x1_{qc}")
              for qc in range(QC)]
        for qc in range(QC):
            for dh in range(2):
                ps = pa.tile([P, T], f32, tag="pa")
                for t in range(4):
                    nc.tensor.matmul(
                        ps, sa_pack[t][:, :, qc * P:(qc + 1) * P],
                        wv_(1, 8192 + dh * 4096 + t * 1024, T),
                        start=(t == 0), stop=False, perf_mode=DR)
                nc.tensor.matmul(ps, r(ones_row[0:1, :]),
                                 cr_sb[0:1, dh * T:(dh + 1) * T],
                                 start=False, stop=True)
                xin = xstr.tile([P, T], f32, tag="xstr")
                nc.gpsimd.dma_start(
                    out=xin, in_=xkv[qc * P:(qc + 1) * P, dh * T:(dh + 1) * T])
                nc.vector.tensor_tensor(x1[qc][:, dh * T:(dh + 1) * T],
                                        ps, xin, ALU.add)

        # ================= Phase D: cross-attention =================
        loadw(3)
        g2x, b2x = lng["ln2x"]
        xp2f = [xsp.tile([P, 2, S], f8, tag="xsp", name=f"xp2_{s}")
                for s in range(4)]
        xp2 = [t[:, :, 0:T] for t in xp2f]
        for qc in range(QC):
            def wr_x2(j, pt, g_c, b_c, qc=qc):
                nc.scalar.activation(
                    xp2[j // 2][:, j % 2, qc * P:(qc + 1) * P],
                    pt, AF.Identity, bias=b_c, scale=g_c)
            ln_tile(x1[qc], g2x, b2x, wr_x2)

        # cq (chunk2 layout [wqc(8192) | wkc(8192)])
        cq_pack = [qp.tile([P, 2, T], f8, tag="qp", name=f"cq_{t}")
                   for t in range(4)]
        for t in range(4):
            for pl in range(2):
                ps = pa.tile([P, T], f32, tag="pa")
                off = (t * 2 + pl) * 1024
                for s in range(4):
                    nc.tensor.matmul(ps, wv_(2, off + s * 256, P), xp2[s],
                                     start=(s == 0), stop=(s == 3),
                                     perf_mode=DR)
                nc.scalar.activation(cq_pack[t][:, pl, :], ps, AF.Identity,
                                     bias=cc_sb[:, CQB0 + t * 2 + pl:
                                                CQB0 + t * 2 + pl + 1])
        # ck over cond tokens
        ck_pack = [ckp.tile([P, 2, SC], f8, tag="ckp", name=f"ck_{t}")
                   for t in range(4)]
        for t in range(4):
            for pl in range(2):
                ps = pa.tile([P, T], f32, tag="pa")
                off = 8192 + (t * 2 + pl) * 1024
                for s in range(4):
                    nc.tensor.matmul(ps[:, 0:SC], wv_(2, off + s * 256, P),
                                     cn_pack[s], start=(s == 0), stop=(s == 3),
                                     perf_mode=DR)
                nc.scalar.activation(ck_pack[t][:, pl, :], ps[:, 0:SC],
                                     AF.Identity,
                                     bias=cc_sb[:, CKB0 + t * 2 + pl:
                                                CKB0 + t * 2 + pl + 1])
        cv_pack = cvp.tile([P, 2, D], f8, tag="cvp")
        for c in range(2):
            for hh in range(2):
                ps = pa.tile([P, T], f32, tag="pa")
                for s in range(4):
                    nc.tensor.matmul(
                        ps, cn_pack[s][:, :, c * P:(c + 1) * P],
                        wv_(3, hh * 4096 + s * 1024, T),
                        start=(s == 0), stop=(s == 3), perf_mode=DR)
                nc.vector.tensor_copy(
                    out=cv_pack[:, c, hh * T:(hh + 1) * T], in_=ps)
        loadw(4)
        loadw(5)

        # cross scores pair, masked exp, sum, AV
        psc = pb.tile([P, 2, T], f32, tag="pb")
        for c in range(2):
            for t in range(4):
                nc.tensor.matmul(
                    psc[:, c, :], ck_pack[t][:, :, c * P:(c + 1) * P],
                    cq_pack[t], start=(t == 0), stop=(t == 3), perf_mode=DR)
        exc = expp.tile([P, 2, T], f8, tag="expp", name="exc")
        nc.scalar.activation(exc, psc, AF.Exp, scale=1.0 / 32.0)
        for c in range(2):
            nc.vector.tensor_scalar_mul(exc[:, c, :], exc[:, c, :],
                                        cc_sb[:, MSK0 + c:MSK0 + c + 1])
        ps_sum = pa.tile([P, T], f32, tag="pa")
        for c in range(2):
            nc.tensor.matmul(ps_sum[0:1, :], ones8[:, c, :], exc[:, c, :],
                             start=(c == 0), stop=(c == 1))
        recip = rcp.tile([1, T], f32r, tag="recip")
        with nc.allow_low_precision(reason="softmax 1/sum in f32r"):
            nc.vector.reciprocal(recip, ps_sum[0:1, :])
        ps_rb = pa.tile([P, T], f32, tag="pa")
        nc.tensor.matmul(ps_rb, r(ones_row), r(recip), start=True, stop=True)
        rb_c = rcp.tile([P, T], f32, tag="rb")
        nc.vector.tensor_copy(out=rb_c, in_=ps_rb)

        cross_pack = [qp.tile([P, 2, T], f8, tag="qp", name=f"cr_{t}")
                      for t in range(4)]
        for jd in range(DT):
            ps = pa.tile([P, T], f32, tag="pa")
            nc.tensor.matmul(ps, cv_pack[:, :, jd * P:(jd + 1) * P], exc,
                             start=True, stop=True, perf_mode=DR)
            nc.vector.tensor_tensor(cross_pack[jd // 2][:, jd % 2, :],
                                    ps, rb_c, ALU.mult)

        # wo proj + bo_eff + gate + residual -> x2 (in place over x1)
        x2 = x1
        for qc in range(QC):
            for dh in range(2):
                ps = pa.tile([P, T], f32, tag="pa")
                for t in range(4):
                    nc.tensor.matmul(
                        ps, cross_pack[t][:, :, qc * P:(qc + 1) * P],
                        wv_(3, 8192 + dh * 4096 + t * 1024, T),
                        start=(t == 0), stop=False, perf_mode=DR)
                nc.tensor.matmul(ps, r(ones_row[0:1, :]),
                                 cr_sb[0:1, D + dh * T:D + (dh + 1) * T],
                                 start=False, stop=True)
                sl = (slice(None), slice(dh * T, (dh + 1) * T))
                nc.vector.scalar_tensor_tensor(
                    x2[qc][sl], ps, cc_sb[:, TG0:TG0 + 1], x1[qc][sl],
                    ALU.mult, ALU.add)

        # ================= Phase E: FFN =================
        g3, b3 = lng["ln3"]
        x3b = [vxp.tile([P, 2080], mybir.dt.uint8, tag="vxp",
                        name=f"x3_{s}") for s in range(4)]
        xs3h = [t[:, 0:1024].bitcast(f8).rearrange("p (pl n) -> p pl n", pl=2)
                for t in x3b]
        xs3l = [t[:, 1024:2048].bitcast(f8e5).rearrange(
                "p (pl n) -> p pl n", pl=2) for t in x3b]
        for qc in range(QC):
            def wr_x3(j, pt, g_c, b_c, qc=qc):
                sc = lsc.tile([P, P], f32, tag="lsc")
                nc.scalar.activation(sc, pt, AF.Identity, bias=b_c,
                                     scale=g_c)
                hi = xs3h[j // 2][:, j % 2, qc * P:(qc + 1) * P]
                nc.gpsimd.tensor_copy(out=hi, in_=sc)
                nc.vector.tensor_tensor(
                    xs3l[j // 2][:, j % 2, qc * P:(qc + 1) * P],
                    sc, hi, ALU.subtract)
            ln_tile(x2[qc], g3, b3, wr_x3)

        loadw2(0)
        # up-proj: 3-chain DR (hi*whi + lo*whi + hi*wlo) + gelu -> h bf16
        h_packb = [khp.tile([P, 2048], mybir.dt.uint8, tag="khp",
                            name=f"h_{t}") for t in range(16)]
        h_pack = [t[:, 0:1024].bitcast(f8).rearrange(
                  "p (pl n) -> p pl n", pl=2) for t in h_packb]
        for f in range(FT):
            half, fi = f // 16, f % 16
            ch_hi, ch_lo = 4 + 2 * half, 5 + 2 * half
            if f == 16:
                loadw(6), loadw(7), loadw2(1)

            ps = pa.tile([P, T], f32, tag="pa")
            off = fi * 1024
            for s in range(4):
                nc.tensor.matmul(ps, wv_(ch_hi, off + s * 256, P), xs3h[s],
                                 start=(s == 0), stop=False, perf_mode=DR)
            for s in range(4):
                nc.tensor.matmul(ps, wv_(ch_hi, off + s * 256, P), xs3l[s],
                                 start=False, stop=False, perf_mode=DR)
            for s in range(4):
                nc.tensor.matmul(ps, wv5_(ch_lo, off + s * 256, P), xs3h[s],
                                 start=False, stop=(s == 3), perf_mode=DR)
            nc.scalar.activation(h_pack[f // 2][:, f % 2, :], ps, gelu_func,
                                 bias=cc_sb[:, B1_0 + f:B1_0 + f + 1])

        # down-proj (bf16) + b2 + residual -> out
        # 8 live psums: dh=0 -> 4 pa tiles, dh=1 -> planes of 2 pb tiles
        ps0 = [pa.tile([P, T], f32, tag="pa", name=f"dps_{qc}")
               for qc in range(QC)]
        psb = [pb.tile([P, 2, T], f32, tag="pb", name=f"dpsb_{i}")
               for i in range(2)]

        def dps(dh, qc):
            return ps0[qc] if dh == 0 else psb[qc // 2][:, qc % 2, :]

        # chunk order hi-dh0 -> hi-dh1 -> lo-dh0 (then drain all dh0
        # groups) -> lo-dh1 (drain dh1 groups): dh0 stores overlap dh1 math
        def drain(dh, qc):
            nc.tensor.matmul(dps(dh, qc), r(ones_row[0:1, :]),
                             cr_sb[0:1, 2 * D + dh * T:2 * D + (dh + 1) * T],
                             start=False, stop=True)
            ot = osb.tile([P, T], f32, tag="osb")
            nc.vector.tensor_tensor(
                ot, dps(dh, qc), x2[qc][:, dh * T:(dh + 1) * T], ALU.add)
            nc.sync.dma_start(
                out=out[qc * P:(qc + 1) * P, dh * T:(dh + 1) * T], in_=ot)

        for dh in range(2):
            for t16 in range(FT // 2):
                for qc in range(QC):
                    nc.tensor.matmul(
                        dps(dh, qc),
                        h_pack[t16][:, :, qc * P:(qc + 1) * P],
                        w2v_(t16, dh, False), start=(t16 == 0), stop=False,
                        perf_mode=DR)
            loadw2(2 + dh)
        for dh in range(2):
            for qc in range(QC):
                for t16 in range(FT // 2):
                    nc.tensor.matmul(
                        dps(dh, qc),
                        h_pack[t16][:, :, qc * P:(qc + 1) * P],
                        w2v_(t16, dh, True), start=False, stop=False,
                        perf_mode=DR)
                drain(dh, qc)

    if compile_hw:
        nc.compile()
    return nc


def _qperm():
    # q_pack/k_pack column order: block (j, pl), col c -> output feature
    idx = np.zeros((4, 2, P), np.int64)
    for j in range(4):
        for pl in range(2):
            for c in range(P):
                idx[j, pl, c] = (4 * j + c // 32) * 64 + pl * 32 + c % 32
    return idx.reshape(-1)


def _dr_img(Wout_in, col_order, np_dt):
    """DR lhsT/rhs image: Wout_in [ncols_total?, 1024] rows=outputs.
    Returns [128, nblk*4s*2dpl*blk] with element [dp, blk, s, dpl, c] =
    W[col_order[blk*blk_sz + c], (2s+dpl)*128 + dp]."""
    W = np.asarray(Wout_in, np.float32)[col_order]      # [ncols, 1024]
    ncols = W.shape[0]
    # -> [dp, s, dpl, col]
    img = W.T.reshape(4, 2, P, ncols).transpose(2, 0, 1, 3)
    return np.ascontiguousarray(img).astype(np_dt)


def _img_cols(img, ncols):
    # [128, 4, 2, ncols] -> flat [128, 4*2*ncols] in (s, dpl, c) order per
    # 128-col block: blocks of 1024 = s*256 + dpl*128 + c
    n = img.shape[3]
    out = np.zeros((P, 0), img.dtype)
    segs = []
    for b0 in range(0, n, ncols):
        blk = img[:, :, :, b0:b0 + ncols]               # [128,4,2,ncols]
        segs.append(blk.reshape(P, -1))
    return np.concatenate(segs, axis=1)


def _pack_dr(Wout_in, col_order, np_dt, blk):
    img = _dr_img(Wout_in, col_order, np_dt)
    return _img_cols(img, blk)


def make_in_maps(inputs):
    f = np.float32
    x = np.asarray(inputs["x"], f)
    cond = np.asarray(inputs["cond"], f)
    cmask = np.asarray(inputs["cond_mask"])
    g = lambda k: np.asarray(inputs[k], f)

    sa_in_w = g("sa_in_w")          # [3D, D]
    sa_out_w = g("sa_out_w")        # [D, D]
    wq, wk_, wv_w, wo = g("wq"), g("wk"), g("wv"), g("wo")
    w1, w2 = g("w1"), g("w2")       # [FF,D], [D,FF]

    qperm = _qperm()
    natural = np.arange(1024)

    wq_img = _pack_dr(sa_in_w[0:D], qperm, NP_F8, P)          # [128, 8192]
    wk_img = _pack_dr(sa_in_w[D:2 * D], qperm, NP_F8, P)
    wv_img = _pack_dr(sa_in_w[2 * D:3 * D], natural, NP_F8, T)
    wso_img = _pack_dr(sa_out_w, natural, NP_F8, T)
    wqc_img = _pack_dr(wq, natural, NP_F8, P)
    wkc_img = _pack_dr(wk_, natural, NP_F8, P)
    wvc_img = _pack_dr(wv_w, natural, NP_F8, T)
    wo_img = _pack_dr(wo, natural, NP_F8, T)

    # w1 hi/lo: [FF, D] rows=ff outputs; blocks of 128 cols
    w1hi = w1.astype(NP_F8).astype(f)
    w1lo = (w1 - w1hi)
    w1hi_img = _pack_dr(w1hi, np.arange(FF), NP_F8, P)        # [128, 32768]
    w1lo_img = _pack_dr(w1lo, np.arange(FF), NP_F8E5, P)

    wall = np.concatenate(
        [wq_img.view(np.uint8), wk_img.view(np.uint8), wv_img.view(np.uint8),
         wso_img.view(np.uint8), wqc_img.view(np.uint8),
         wkc_img.view(np.uint8), wvc_img.view(np.uint8), wo_img.view(np.uint8),
         w1hi_img[:, 0:16384].view(np.uint8),
         np.ascontiguousarray(w1lo_img[:, 0:16384]).view(np.uint8),
         w1hi_img[:, 16384:32768].view(np.uint8),
         np.ascontiguousarray(w1lo_img[:, 16384:32768]).view(np.uint8)],
        axis=1).view(NP_F8)
    wch = [wall[:, 8192 * i:8192 * (i + 1)] for i in range(16)]

    # w2 fp8 DR rhs: [dp, t16, pl, c] = w2[c, (2*t16+pl)*128+dp]
    # [p, t16, pl, dh, c]: chunk (hi/lo, dh) holds [t16, pl, 512] per dh
    w2T = w2.T.reshape(16, 2, P, 2, T).transpose(2, 0, 1, 3, 4)
    w2f = np.ascontiguousarray(w2T)                 # [128,16,2,2,512]
    w2hi = w2f.astype(NP_F8)
    w2lo = (w2f - w2hi.astype(np.float32)).astype(NP_F8E5)
    def _dhc(a, dh):
        return np.ascontiguousarray(a[:, :, :, dh, :]).reshape(P, 16384)
    w2ch = [_dhc(w2hi, 0), _dhc(w2hi, 1),
            _dhc(w2lo, 0).view(np.uint8).view(NP_F8),
            _dhc(w2lo, 1).view(np.uint8).view(NP_F8)]

    # consts
    ccol = np.zeros((P, NCOL), f)
    qb = g("sa_in_b")[0:D][qperm].reshape(8, P).T             # [(j,pl) cols]
    kb = g("sa_in_b")[D:2 * D][qperm].reshape(8, P).T
    ccol[:, QB0:QB0 + 8] = qb
    ccol[:, KB0:KB0 + 8] = kb
    ccol[:, B1_0:B1_0 + 32] = g("b1").reshape(32, P).T
    ccol[:, CQB0:CQB0 + 8] = g("bq").reshape(8, P).T
    ccol[:, CKB0:CKB0 + 8] = g("bk").reshape(8, P).T
    ccol[:, TG0] = np.tanh(g("gate"))[0]
    for i, nm in enumerate(("ln1", "ln2x", "ln2c", "ln3")):
        gk = {"ln1": "ln1_g", "ln2x": "ln2x_g",
              "ln2c": "ln2c_g", "ln3": "ln3_g"}[nm]
        bk = gk[:-2] + "_b"
        ccol[:, LN0 + 2 * i * 8:LN0 + 2 * i * 8 + 8] = g(gk).reshape(8, P).T
        ccol[:, LN0 + (2 * i + 1) * 8:LN0 + (2 * i + 1) * 8 + 8] = \
            g(bk).reshape(8, P).T

    crow = np.zeros((1, 3 * D), f)
    crow[0, 0:D] = g("sa_out_b") + g("sa_in_b")[2 * D:3 * D] @ sa_out_w.T
    crow[0, D:2 * D] = g("bo") + g("bv") @ wo.T
    crow[0, 2 * D:3 * D] = g("b2")

    shared = {f"wch{i}": np.ascontiguousarray(wch[i]) for i in range(16)}
    shared.update({f"w2ch{i}": np.ascontiguousarray(w2ch[i])
                   for i in range(4)})
    shared["crow"] = crow

    in_maps = []
    for c in range(N_CORES):
        b, half = c // 2, c % 2
        loc = x[b, half * T:(half + 1) * T]
        oth = x[b, (1 - half) * T:(2 - half) * T]
        m = dict(shared)
        m["xkv"] = np.ascontiguousarray(loc)
        m["xoth"] = np.ascontiguousarray(oth.astype(NP_BF16))
        m["cond"] = np.ascontiguousarray(cond[b].astype(NP_BF16))
        cc = ccol.copy()
        cc[:, MSK0:MSK0 + 2] = \
            (cmask[b] != 0).astype(f).reshape(2, P).T
        m["ccol"] = cc
        in_maps.append(m)
    return in_maps


_CACHED_NC = None


def kernel(**inputs):
    from concourse.bass_utils import run_bass_kernel_spmd
    global _CACHED_NC
    if _CACHED_NC is None:
        _CACHED_NC = build_nc(compile_hw=True)
    in_maps = make_in_maps(inputs)
    res = run_bass_kernel_spmd(_CACHED_NC, in_maps, list(range(N_CORES)))
    out = np.empty((B, S, D), np.float32)
    for c in range(N_CORES):
        b, half = c // 2, c % 2
        out[b, half * T:(half + 1) * T] = res.results[c]["out"]
    return out
